# revision 19
# baseline (speedup 1.0000x reference)
"""Ragged segment self-attention (AttentionHiddenNet) on 8 Trainium2 cores.

Fixed problem instance: h_states [1, 163840, 64] fp32, 4096 segments whose
lengths cycle through [16, 24, 32, 40, 48, 56, 64, 40] (320 tokens / cycle).
Per segment s: ctx_s = softmax(H_s @ H_s^T, axis=-1) @ H_s.

Sharding: 512 consecutive segments (= 64 cycles = 20480 tokens, contiguous
rows) per core; no cross-core communication.

Per-core algorithm: consecutive segments are packed into "groups" of <= 128
tokens (per 320-token cycle: [16,24,32,40] -> 112, [48,56] -> 104,
[64,40] -> 104).  For each group (dense padded scores):
    S' = G^T G  where G = [H_g^T ; W_g]  (PE, fp16, K = 64 + 4)
         W_g rows carry 30.0 on each segment's token range, so
         S'[l,m] = h_l.h_m + 900 * same_segment(l,m)
    U  = exp(S' - 1000)           (ACT, bias = -1000; off-segment pairs
                                   underflow to exactly 0 = the mask,
                                   on-segment shift is -100 overflow guard)
    C|Z = U^T @ [H_g | 1]         (PE, bf16; U symmetric per block)
    out = C * (1/Z)               (reciprocal on DVE; the broadcast multiply
                                   alternates DVE / GpSimd to balance load)
Four consecutive cycles of a group type share one PSUM bank / one exp /
one normalize, cutting ACT+DVE instruction counts 4x.  All HBM traffic is
16-bit, packed on host so every DMA line is a contiguous >=2KB run, and the
three group types ride in one padded [112, ...] tensor per direction (12
DMA instructions total).  The host unpacks the grouped bf16 output back to
token order in fp32.
"""

import numpy as np

H_DIM = 64
NUM_SEQS = 4096
LEN_PATTERN = [16, 24, 32, 40, 48, 56, 64, 40]
N_TOTAL = 163840
N_CORES = 8
SEGS_PER_CORE = NUM_SEQS // N_CORES          # 512
CYCLE_TOKS = sum(LEN_PATTERN)                # 320
CYCLES_PER_CORE = SEGS_PER_CORE // len(LEN_PATTERN)   # 64
TOKS_PER_CORE = CYCLES_PER_CORE * CYCLE_TOKS          # 20480

# groups of consecutive segments, <= 128 tokens each: (tok_off, L, lens)
GROUP_TYPES = [
    (0, 112, (16, 24, 32, 40)),
    (112, 104, (48, 56)),
    (216, 104, (64, 40)),
]
NTYPES = len(GROUP_TYPES)
LMAX = 112                    # padded partition count of the packed tensors
MASK_ROWS = 4                 # max segments per group
KDIM = H_DIM + MASK_ROWS      # 68
NEG_SHIFT = -1000.0           # exp bias (off-segment: exp underflows to 0)
W_MASK = 30.0                 # 30^2 = 900 on-segment; -1000+900 = -100 guard

# chunk sizes in cycles (each a multiple of 8 so octets tile evenly);
# small first chunk = early compute start, small last chunk = short tail
import os as _os
CHUNK_CYCLES = [int(x) for x in _os.environ.get("KCHUNKS", "8,16,8,8,8,8,8").split(",")]
CYC_PER_QUAD = 4

_CACHE = {}
LAST_RESULT = None


def _expected_sse():
    lens = np.tile(np.array(LEN_PATTERN, dtype=np.int64), NUM_SEQS // len(LEN_PATTERN))
    ends = np.cumsum(lens)
    starts = np.concatenate([[0], ends[:-1]])
    return np.stack([starts, ends], axis=1)


def _build_bass():
    import concourse.bass as bass
    import concourse.bacc as bacc
    import concourse.tile as tile
    from concourse import mybir
    from contextlib import ExitStack

    f32 = mybir.dt.float32
    f16 = mybir.dt.float16
    bf16 = mybir.dt.bfloat16

    nc = bacc.Bacc("TRN2")
    ht_d = nc.dram_tensor("ht", [KDIM, TOKS_PER_CORE], f16, kind="ExternalInput")
    hg_d = nc.dram_tensor(
        "hg", [LMAX, CYCLES_PER_CORE, NTYPES, H_DIM + 2], bf16, kind="ExternalInput"
    )
    og_d = nc.dram_tensor(
        "og", [LMAX, CYCLES_PER_CORE, NTYPES, H_DIM], bf16, kind="ExternalOutput"
    )

    with tile.TileContext(nc) as tc, ExitStack() as ctx:
        singles = ctx.enter_context(tc.tile_pool(name="singles", bufs=1))
        htpool = ctx.enter_context(tc.tile_pool(name="htpool", bufs=2))
        hgpool = ctx.enter_context(tc.tile_pool(name="hgpool", bufs=2))
        upool = ctx.enter_context(tc.tile_pool(name="upool", bufs=3))
        rpool = ctx.enter_context(tc.tile_pool(name="rpool", bufs=4))
        ps_s = ctx.enter_context(tc.tile_pool(name="ps_s", bufs=2, space="PSUM"))
        ps_c = ctx.enter_context(tc.tile_pool(name="ps_c", bufs=2, space="PSUM"))

        bias_t = singles.tile([128, 1], f32)
        nc.vector.memset(bias_t[:, :], NEG_SHIFT)

        hg_row = NTYPES * (H_DIM + 2)       # per-cycle free elems of hg
        og_row = NTYPES * H_DIM             # per-cycle free elems of og

        # manual double-buffer for og so the padding rows (104:112 of the
        # t1/t2 slots, shipped but ignored by the host) can be initialized
        # once per physical buffer instead of per chunk
        max_cyc = max(CHUNK_CYCLES)
        og_bufs = []
        for i in range(2):
            ogb = singles.tile([LMAX, max_cyc, NTYPES, H_DIM], bf16,
                               name=f"ogb{i}")
            nc.gpsimd.memset(ogb[96:LMAX, :, 1:NTYPES, :], 0.0)
            og_bufs.append(ogb)

        cyc0 = 0
        for ck, ncyc in enumerate(CHUNK_CYCLES):
            tok0 = cyc0 * CYCLE_TOKS
            ntoks = ncyc * CYCLE_TOKS
            ht_k = htpool.tile([KDIM, max_cyc * CYCLE_TOKS], f16, tag="ht")
            nc.sync.dma_start(
                ht_k[:, 0:ntoks],
                bass.AP(ht_d, tok0, [[TOKS_PER_CORE, KDIM], [1, ntoks]]),
            )
            hg = hgpool.tile([LMAX, max_cyc, NTYPES, H_DIM + 2], bf16, tag="hg")
            nc.sync.dma_start(
                hg[:, 0:ncyc, :, :],
                bass.AP(
                    hg_d,
                    cyc0 * hg_row,
                    [[CYCLES_PER_CORE * hg_row, LMAX], [1, ncyc * hg_row]],
                ),
            )
            og = og_bufs[ck % 2]

            for oc in range(ncyc // 8):
                for t, (off, L, _lens) in enumerate(GROUP_TYPES):
                    # 8 cycles of scores, blocks at stride 128 so each
                    # matmul output stays within one PSUM bank
                    s_ps = ps_s.tile([128, 8, 128], f32, tag="s")
                    for c in range(8):
                        cyc = oc * 8 + c
                        ktok = cyc * CYCLE_TOKS + off
                        g = ht_k[:, ktok : ktok + L]
                        nc.tensor.matmul(
                            s_ps[0:L, c, 0:L],
                            g, g, start=True, stop=True,
                        )
                    u = upool.tile([128, 8, 128], bf16, tag="u")
                    nc.scalar.activation(
                        u[0:L, 0:8, 0:L],
                        s_ps[0:L, 0:8, 0:L],
                        mybir.ActivationFunctionType.Exp,
                        bias=bias_t[0:L, :],
                    )
                    # C for all 8 cycles, blocks at stride 128 (2 PSUM banks)
                    c_ps = ps_c.tile([128, 8, 128], f32, tag="c")
                    for c in range(8):
                        cyc = oc * 8 + c
                        nc.tensor.matmul(
                            c_ps[0:L, c, 0 : H_DIM + 2],
                            u[0:L, c, 0:L],
                            hg[0:L, cyc, t, :],
                            start=True, stop=True,
                        )
                    r = rpool.tile([128, 8], f32, tag="r")
                    nc.vector.reciprocal(r[0:L, :], c_ps[0:L, 0:8, H_DIM])
                    qc0 = oc * 8
                    nc.vector.tensor_tensor(
                        og[0:L, qc0 : qc0 + 8, t, :],
                        c_ps[0:L, 0:8, 0:H_DIM],
                        r[0:L, :].broadcast_to((L, 8, H_DIM)),
                        mybir.AluOpType.mult,
                    )

            # store from the gpsimd queue so it never blocks SP's load prefetch
            nc.gpsimd.dma_start(
                bass.AP(
                    og_d,
                    cyc0 * og_row,
                    [[CYCLES_PER_CORE * og_row, LMAX], [1, ncyc * og_row]],
                ),
                og[:, 0:ncyc, :, :],
            )
            cyc0 += ncyc

    nc.compile()
    return nc


def _make_core_inputs(slab):
    """slab: [20480, 64] f32 -> input map for one core."""
    import ml_dtypes

    bf16 = ml_dtypes.bfloat16
    ht = np.zeros((KDIM, TOKS_PER_CORE), dtype=np.float16)
    ht[0:H_DIM] = slab.T.astype(np.float16)
    # periodic mask rows: row 64+gi = 30.0 over segment gi of each group
    pat = np.zeros((MASK_ROWS, CYCLE_TOKS), dtype=np.float16)
    for off, L, lens in GROUP_TYPES:
        p = off
        for gi, ln in enumerate(lens):
            pat[gi, p : p + ln] = W_MASK
            p += ln
    ht[H_DIM:] = np.tile(pat, (1, CYCLES_PER_CORE))

    cyc_base = np.arange(CYCLES_PER_CORE) * CYCLE_TOKS
    hg = np.zeros((LMAX, CYCLES_PER_CORE, NTYPES, H_DIM + 2), dtype=bf16)
    for t, (off, L, _lens) in enumerate(GROUP_TYPES):
        idx = cyc_base[None, :] + off + np.arange(L)[:, None]   # [L, 64]
        hg[0:L, :, t, 0:H_DIM] = slab[idx].astype(bf16)
        hg[0:L, :, t, H_DIM:] = bf16(1.0)
    return {"ht": ht, "hg": hg}


def _unpack_core_output(res_map):
    """res_map: {'og': [112, 64, 3, 64] bf16} -> [20480, 64] f32."""
    og = np.asarray(res_map["og"]).astype(np.float32)
    out = np.empty((TOKS_PER_CORE, H_DIM), dtype=np.float32)
    cyc_base = np.arange(CYCLES_PER_CORE) * CYCLE_TOKS
    for t, (off, L, _lens) in enumerate(GROUP_TYPES):
        idx = cyc_base[None, :] + off + np.arange(L)[:, None]   # [L, 64]
        out[idx.reshape(-1)] = og[0:L, :, t, :].reshape(-1, H_DIM)
    return out


def _run_numpy(h, sse):
    # generic host fallback (only used if the input does not match the
    # hardcoded segment pattern)
    out = np.empty_like(h)
    for s, e in sse:
        seg = h[s:e]
        sc = seg @ seg.T
        sc -= sc.max(axis=-1, keepdims=True)
        u = np.exp(sc)
        out[s:e] = (u / u.sum(axis=-1, keepdims=True)) @ seg
    return out


def kernel(h_states, seq_start_end):
    global LAST_RESULT
    h = np.asarray(h_states, dtype=np.float32).reshape(-1, H_DIM)
    sse = np.asarray(seq_start_end).astype(np.int64)

    if h.shape[0] != N_TOTAL or not np.array_equal(sse, _expected_sse()):
        return _run_numpy(h, sse).astype(np.float32)

    from concourse.bass_utils import run_bass_kernel_spmd

    if "nc" not in _CACHE:
        _CACHE["nc"] = _build_bass()
    nc = _CACHE["nc"]

    in_maps = [
        _make_core_inputs(h[c * TOKS_PER_CORE : (c + 1) * TOKS_PER_CORE])
        for c in range(N_CORES)
    ]
    res = run_bass_kernel_spmd(nc, in_maps, core_ids=list(range(N_CORES)))
    LAST_RESULT = res
    out = np.concatenate([_unpack_core_output(r) for r in res.results], axis=0)
    return out.astype(np.float32)
